# revision 10
# baseline (speedup 1.0000x reference)
"""DTLN part-2 single streaming step on 8 TRN2 NeuronCores.

Structure of the reference:
    enc = enc_W @ y1                         # [256] matvec, 1MB weights
    instant-LN(enc) -> LSTM1 -> LSTM2        # 1.4MB weights, fully sequential
    mask = sigmoid(dense_W @ h2 + b)         # 128KB
    decoded = dec_W @ (mask * enc)           # [1024] matvec, 1MB weights

Sharding: the middle is sequential (global LN stats + LSTM state), so it is
replicated on every core.  Only the final dec_W matvec shards with zero
communication: core k owns output rows [128k, 128k+128) and the host
concatenates.  Per-core HBM->SBUF traffic drops from 3.4MB to 2.62MB.

On-device layout: every vector is a column ([d<=128, 1] per tile, elements on
partitions).  All matvecs are PE matmuls with the weight tile as the
stationary operand (lhsT = W.T tile) and the activation column as the N=1
moving operand; fp32 moving operands stream at 1/4 rate so the weights must
be stationary.  The instant-LN is algebraically dissolved:

    z1 = rstd*(Wih1g @ enc) + (-rstd*mean)*(Wih1g @ 1) + [Wih1@beta + b1] + Whh1@h1
         (Wih1g = Wih1 * gamma broadcast over columns; first term starts
          before the LN statistics are even computed)

Cross-partition reductions (mean, sum-of-squares) use a ones[128,128] matmul
which reduces AND broadcasts in one PE instruction.  ACT table thrash is
avoided: Sqrt table is preloaded at t=0 under the weight DMA, and the
Sigmoid/Tanh table (one set) loads right after the LN sqrt, hidden under the
LSTM1 matmuls.
"""

import numpy as np

FRAME = 1024
ENC = 256
HID = 128
EPS = 1e-7
NCORES = 8
P = 128

# gate order i, f, o, g (PyTorch layout is i, f, g, o)
_PERM = np.concatenate([np.arange(0, 128), np.arange(128, 256),
                        np.arange(384, 512), np.arange(256, 384)])

NV = 26  # columns in the small-vector blob

_CACHE: dict = {}


def _build_bass():
    import concourse.bacc as bacc
    import concourse.mybir as mybir
    import concourse.tile as tile

    f32 = mybir.dt.float32
    AFT = mybir.ActivationFunctionType
    ALU = mybir.AluOpType

    nc = bacc.Bacc()

    # DRAM I/O.  Weight blobs are pre-transposed/packed on the host so that
    # each [128, 128*m] slice is directly a matmul lhsT tile.
    d_vecs = nc.dram_tensor("vecs", [P, NV], f32, kind="ExternalInput")
    d_we0 = nc.dram_tensor("we0", [P, 1024], f32, kind="ExternalInput")
    d_we1 = nc.dram_tensor("we1", [P, 1024], f32, kind="ExternalInput")
    d_wz1 = nc.dram_tensor("wz1", [P, 1536], f32, kind="ExternalInput")
    d_wz2 = nc.dram_tensor("wz2", [P, 1024], f32, kind="ExternalInput")
    d_wtl = nc.dram_tensor("wtl", [P, 512], f32, kind="ExternalInput")
    d_hc = nc.dram_tensor("hc", [P, 4], f32, kind="ExternalOutput")
    d_dec = nc.dram_tensor("dec", [P, 1], f32, kind="ExternalOutput")

    with tile.TileContext(nc) as tc:
        with (
            tc.tile_pool(name="w", bufs=1) as wp,
            tc.tile_pool(name="s", bufs=1) as sp,
            tc.tile_pool(name="ps", bufs=1, space="PSUM") as pp,
        ):
            vecs = wp.tile([P, NV], f32, tag="vecs")
            we0 = wp.tile([P, 1024], f32, tag="we0")
            we1 = wp.tile([P, 1024], f32, tag="we1")
            wz1 = wp.tile([P, 1536], f32, tag="wz1")
            wz2 = wp.tile([P, 1024], f32, tag="wz2")
            wtl = wp.tile([P, 512], f32, tag="wtl")

            # Stream weights in stage order; separate tiles so matmuls wait
            # only on the chunk they read.
            nc.sync.dma_start(vecs[:], d_vecs[:])
            nc.sync.dma_start(we0[:], d_we0[:])
            nc.sync.dma_start(we1[:], d_we1[:])
            nc.sync.dma_start(wz1[:], d_wz1[:])
            nc.sync.dma_start(wz2[:], d_wz2[:])
            nc.sync.dma_start(wtl[:], d_wtl[:])

            ones = sp.tile([P, P], f32, tag="ones")
            nc.gpsimd.memset(ones[:], 1.0)
            konst = sp.tile([1, 1], f32, tag="konst")
            nc.gpsimd.memset(konst[:], 1.0)
            junk1 = sp.tile([1, 1], f32, tag="junk1")
            junk2 = sp.tile([P, 1], f32, tag="junk2")

            # Preload the Sqrt ACT table while DMA streams (ACT is idle).
            nc.scalar.activation(junk1[:], konst[:], AFT.Sqrt)

            # The LDWEIGHTS ISA struct has a single sync-wait slot, so every
            # Matmult may carry at most ONE semaphore wait.  "Gate" matmuls
            # (1x1 junk output, WAW-ordered into the group's PSUM tile)
            # absorb one new semaphore each into the PE's vector clock just
            # before each matmul group.
            def gate(ps_tile, src):
                nc.tensor.matmul(ps_tile[0:1, 0:1], src[:, 0:1], src[:, 0:1],
                                 start=True, stop=True)

            # ---- encoder: enc[256] = enc_W @ y1, columns of enc_ps ----
            enc_ps = pp.tile([P, 2], f32, tag="enc_ps")
            gate(enc_ps, vecs)          # absorbs the vecs DMA sem
            for m in range(2):
                for kk in range(8):
                    w = we0 if kk < 4 else we1
                    c = (kk % 4) * 256 + 128 * m
                    nc.tensor.matmul(enc_ps[:, m:m + 1], w[:, c:c + 128],
                                     vecs[:, kk:kk + 1],
                                     start=(kk == 0), stop=(kk == 7))
            enc_sb = sp.tile([P, 2], f32, tag="enc_sb")
            nc.vector.tensor_copy(enc_sb[:], enc_ps[:])

            # ---- LN statistics: sum and sum-of-squares via ones-matmul ----
            sq = sp.tile([P, 2], f32, tag="sq")
            nc.vector.tensor_mul(sq[:], enc_sb[:], enc_sb[:])
            st_ps = pp.tile([P, 2], f32, tag="st_ps")
            gate(st_ps, ones)           # absorbs the ones-memset Pool sem
            for kk in range(2):
                nc.tensor.matmul(st_ps[:, 0:1], ones[:], enc_sb[:, kk:kk + 1],
                                 start=(kk == 0), stop=(kk == 1))
            for kk in range(2):
                nc.tensor.matmul(st_ps[:, 1:2], ones[:], sq[:, kk:kk + 1],
                                 start=(kk == 0), stop=(kk == 1))
            mean = sp.tile([P, 1], f32, tag="mean")
            nc.scalar.activation(mean[:], st_ps[:, 0:1], AFT.Identity,
                                 scale=1.0 / ENC)
            ssqe = sp.tile([P, 1], f32, tag="ssqe")
            nc.scalar.activation(ssqe[:], st_ps[:, 1:2], AFT.Identity,
                                 scale=1.0 / ENC)
            # m2e = mean^2 - EPS, so ve = E[x^2] - mean^2 + EPS
            m2 = sp.tile([P, 1], f32, tag="m2")
            nc.vector.tensor_scalar(m2[:], mean[:], mean[:], -EPS,
                                    ALU.mult, ALU.add)
            ve = sp.tile([P, 1], f32, tag="ve")
            nc.vector.tensor_sub(ve[:], ssqe[:], m2[:])
            std = sp.tile([P, 1], f32, tag="std")
            nc.scalar.activation(std[:], ve[:], AFT.Sqrt)
            # Kick the Sigmoid/Tanh table load now (depends on std only so it
            # lands right after the sqrt, hidden under the LSTM1 matmuls).
            nc.scalar.activation(junk2[:], std[:], AFT.Sigmoid)
            rstd = sp.tile([P, 1], f32, tag="rstd")
            nc.vector.reciprocal(rstd[:], std[:])
            ns = sp.tile([P, 1], f32, tag="ns")
            nc.vector.tensor_scalar(ns[:], mean[:], rstd[:], -1.0,
                                    ALU.mult, ALU.mult)
            bias1 = sp.tile([P, 4], f32, tag="bias1")
            nc.vector.tensor_scalar(bias1[:], vecs[:, 20:24], ns[:], None,
                                    ALU.mult)
            bias1b = sp.tile([P, 4], f32, tag="bias1b")
            nc.vector.tensor_tensor(bias1b[:], bias1[:], vecs[:, 12:16],
                                    ALU.add)

            # ---- LSTM1: u1 = Wih1g@enc (needs only raw enc), v1 = Whh1@h1 ----
            u1_ps = pp.tile([P, 4], f32, tag="u1_ps")
            gate(u1_ps, wz1)            # absorbs the wz1 DMA sem
            for m in range(4):
                for kk in range(2):
                    c = 512 * kk + 128 * m
                    nc.tensor.matmul(u1_ps[:, m:m + 1], wz1[:, c:c + 128],
                                     enc_sb[:, kk:kk + 1],
                                     start=(kk == 0), stop=(kk == 1))
            v1_ps = pp.tile([P, 4], f32, tag="v1_ps")
            for m in range(4):
                c = 1024 + 128 * m
                nc.tensor.matmul(v1_ps[:, m:m + 1], wz1[:, c:c + 128],
                                 vecs[:, 8:9], start=True, stop=True)
            t0 = sp.tile([P, 4], f32, tag="t0")
            nc.vector.tensor_scalar(t0[:], u1_ps[:], rstd[:], None, ALU.mult)
            t1 = sp.tile([P, 4], f32, tag="t1")
            nc.vector.tensor_tensor(t1[:], t0[:], bias1b[:], ALU.add)
            zin1 = sp.tile([P, 4], f32, tag="zin1")
            nc.vector.tensor_tensor(zin1[:], t1[:], v1_ps[:], ALU.add)
            g1 = sp.tile([P, 4], f32, tag="g1")
            nc.scalar.activation(g1[:, 0:3], zin1[:, 0:3], AFT.Sigmoid)
            nc.scalar.activation(g1[:, 3:4], zin1[:, 3:4], AFT.Tanh)

            hc = sp.tile([P, 4], f32, tag="hc")  # h1, c1, h2, c2 columns
            p1 = sp.tile([P, 1], f32, tag="p1")
            nc.vector.tensor_mul(p1[:], g1[:, 0:1], g1[:, 3:4])
            # c1_new = c1_in * f + i*g
            nc.vector.tensor_scalar(hc[:, 1:2], vecs[:, 9:10], g1[:, 1:2],
                                    p1[:], ALU.mult, ALU.add)
            tc1 = sp.tile([P, 1], f32, tag="tc1")
            nc.scalar.activation(tc1[:], hc[:, 1:2], AFT.Tanh)
            nc.vector.tensor_mul(hc[:, 0:1], g1[:, 2:3], tc1[:])

            # ---- LSTM2: v2 = Whh2@h2_in runs early; u2 = Wih2@h1 ----
            v2_ps = pp.tile([P, 4], f32, tag="v2_ps")
            gate(v2_ps, wz2)            # absorbs the wz2 DMA sem
            for m in range(4):
                c = 512 + 128 * m
                nc.tensor.matmul(v2_ps[:, m:m + 1], wz2[:, c:c + 128],
                                 vecs[:, 10:11], start=True, stop=True)
            e2 = sp.tile([P, 4], f32, tag="e2")
            nc.vector.tensor_tensor(e2[:], v2_ps[:], vecs[:, 16:20], ALU.add)
            u2_ps = pp.tile([P, 4], f32, tag="u2_ps")
            for m in range(4):
                nc.tensor.matmul(u2_ps[:, m:m + 1], wz2[:, 128 * m:128 * m + 128],
                                 hc[:, 0:1], start=True, stop=True)
            zin2 = sp.tile([P, 4], f32, tag="zin2")
            nc.vector.tensor_tensor(zin2[:], u2_ps[:], e2[:], ALU.add)
            g2 = sp.tile([P, 4], f32, tag="g2")
            nc.scalar.activation(g2[:, 0:3], zin2[:, 0:3], AFT.Sigmoid)
            nc.scalar.activation(g2[:, 3:4], zin2[:, 3:4], AFT.Tanh)
            p2 = sp.tile([P, 1], f32, tag="p2")
            nc.vector.tensor_mul(p2[:], g2[:, 0:1], g2[:, 3:4])
            nc.vector.tensor_scalar(hc[:, 3:4], vecs[:, 11:12], g2[:, 1:2],
                                    p2[:], ALU.mult, ALU.add)
            tc2 = sp.tile([P, 1], f32, tag="tc2")
            nc.scalar.activation(tc2[:], hc[:, 3:4], AFT.Tanh)
            nc.vector.tensor_mul(hc[:, 2:3], g2[:, 2:3], tc2[:])

            nc.sync.dma_start(d_hc[:], hc[:])

            # ---- dense mask + decoder shard ----
            d_ps = pp.tile([P, 2], f32, tag="d_ps")
            gate(d_ps, wtl)             # absorbs the wtl DMA sem
            for m in range(2):
                nc.tensor.matmul(d_ps[:, m:m + 1], wtl[:, 128 * m:128 * m + 128],
                                 hc[:, 2:3], start=True, stop=True)
            msk = sp.tile([P, 2], f32, tag="msk")
            for m in range(2):
                nc.scalar.activation(msk[:, m:m + 1], d_ps[:, m:m + 1],
                                     AFT.Sigmoid, bias=vecs[:, 24 + m:25 + m])
            est = sp.tile([P, 2], f32, tag="est")
            nc.vector.tensor_mul(est[:], msk[:], enc_sb[:])
            o_ps = pp.tile([P, 1], f32, tag="o_ps")
            for kk in range(2):
                c = 256 + 128 * kk
                nc.tensor.matmul(o_ps[:, 0:1], wtl[:, c:c + 128],
                                 est[:, kk:kk + 1],
                                 start=(kk == 0), stop=(kk == 1))
            dec_sb = sp.tile([P, 1], f32, tag="dec_sb")
            nc.vector.tensor_copy(dec_sb[:], o_ps[:])
            nc.sync.dma_start(d_dec[:], dec_sb[:])

    nc.compile()
    return nc


def _pack_inputs(inputs):
    """Host-side packing: transpose/permute weights into lhsT tile blobs."""
    f = lambda x: np.ascontiguousarray(np.asarray(x, dtype=np.float32))
    y1 = f(inputs["y1"])
    h1_in, c1_in = f(inputs["h1_in"]), f(inputs["c1_in"])
    h2_in, c2_in = f(inputs["h2_in"]), f(inputs["c2_in"])
    enc_W = f(inputs["enc_W"])
    gamma, beta = f(inputs["gamma"]), f(inputs["beta"])
    Wih1, Whh1 = f(inputs["Wih1"]), f(inputs["Whh1"])
    bih1, bhh1 = f(inputs["bih1"]), f(inputs["bhh1"])
    Wih2, Whh2 = f(inputs["Wih2"]), f(inputs["Whh2"])
    bih2, bhh2 = f(inputs["bih2"]), f(inputs["bhh2"])
    dense_W, dense_b = f(inputs["dense_W"]), f(inputs["dense_b"])
    dec_W = f(inputs["dec_W"])

    G1 = Wih1 * gamma[None, :]
    Pg1 = G1[_PERM]                       # [512, 256] gate-permuted
    Ph1 = Whh1[_PERM]                     # [512, 128]
    Pi2 = Wih2[_PERM]
    Ph2 = Whh2[_PERM]
    c1b = (Wih1 @ beta + bih1 + bhh1)[_PERM]
    c2b = (bih2 + bhh2)[_PERM]
    w1v = Pg1.sum(axis=1)                 # Wih1g @ ones

    vecs = np.zeros((P, NV), np.float32)
    vecs[:, 0:8] = y1.reshape(8, P).T
    vecs[:, 8] = h1_in
    vecs[:, 9] = c1_in
    vecs[:, 10] = h2_in
    vecs[:, 11] = c2_in
    vecs[:, 12:16] = c1b.reshape(4, P).T
    vecs[:, 16:20] = c2b.reshape(4, P).T
    vecs[:, 20:24] = w1v.reshape(4, P).T
    vecs[:, 24:26] = dense_b.reshape(2, P).T

    eT = np.ascontiguousarray(enc_W.T).reshape(8, P, ENC)  # k-tiles
    we0 = np.concatenate([eT[i] for i in range(4)], axis=1)
    we1 = np.concatenate([eT[i] for i in range(4, 8)], axis=1)

    g1T = np.ascontiguousarray(Pg1.T).reshape(2, P, 512)
    wz1 = np.concatenate([g1T[0], g1T[1], Ph1.T], axis=1)  # [128, 1536]
    wz2 = np.concatenate([Pi2.T, Ph2.T], axis=1)           # [128, 1024]

    in_maps = []
    for k in range(NCORES):
        Dk = dec_W[P * k:P * (k + 1), :]                   # [128, 256]
        dT = np.ascontiguousarray(Dk.T).reshape(2, P, P)
        wtl = np.concatenate([dense_W.T, dT[0], dT[1]], axis=1)  # [128, 512]
        in_maps.append({
            "vecs": vecs,
            "we0": np.ascontiguousarray(we0),
            "we1": np.ascontiguousarray(we1),
            "wz1": np.ascontiguousarray(wz1),
            "wz2": np.ascontiguousarray(wz2),
            "wtl": np.ascontiguousarray(wtl),
        })
    return in_maps


def _get_nc():
    if "nc" not in _CACHE:
        _CACHE["nc"] = _build_bass()
    return _CACHE["nc"]


def kernel(**inputs):
    from concourse.bass_utils import run_bass_kernel_spmd

    nc = _get_nc()
    in_maps = _pack_inputs(inputs)
    res = run_bass_kernel_spmd(nc, in_maps, list(range(NCORES))).results

    decoded = np.concatenate([res[k]["dec"][:, 0] for k in range(NCORES)])
    hc = res[0]["hc"]
    return (
        decoded.reshape(1, FRAME, 1).astype(np.float32),
        hc[:, 0].reshape(1, 1, HID).astype(np.float32),
        hc[:, 1].reshape(1, 1, HID).astype(np.float32),
        hc[:, 2].reshape(1, 1, HID).astype(np.float32),
        hc[:, 3].reshape(1, 1, HID).astype(np.float32),
    )


# revision 31
# speedup vs baseline: 1.7168x; 1.7168x over previous
"""DTLN part-2 single streaming step on 8 TRN2 NeuronCores.

Structure of the reference:
    enc = enc_W @ y1                         # [256] matvec, 1MB weights
    instant-LN(enc) -> LSTM1 -> LSTM2        # 1.4MB weights, fully sequential
    mask = sigmoid(dense_W @ h2 + b)         # 128KB
    decoded = dec_W @ (mask * enc)           # [1024] matvec, 1MB weights

Sharding: the middle is sequential (global LN stats + LSTM state), so it is
replicated on every core.  Only the final dec_W matvec shards with zero
communication: core k owns output rows [128k, 128k+128) and the host
concatenates.

Layout: every vector is a column ([d<=128, 1] per tile, elements on
partitions).  Matvecs are PE matmuls with the weight tile stationary
(lhsT = W.T tile) and the activation column as the N=1 moving operand.
The instant-LN is dissolved algebraically so LSTM1 input matmuls start
straight from raw enc; mean/sum-sq reductions use a ones[128,128] matmul
(reduce + broadcast in one PE op).

Outputs are converted to ROW layout on-chip before the store (dec via an
operand-swapped matmul, h/c via one fp32 identity matmul) — a [128,1]
column DMA fans out into 128 four-byte descriptors whose HBM write
receipts trickle in for ~7us, while a [1,128] row is one descriptor.

ACT tables: Sqrt is preloaded at t=0 (hidden under the DMA); the
Sigmoid/Tanh load is kicked right after the LN sqrt, under the LSTM1
matmuls.  Small inputs go on the scalar-engine HWDGE ring so the weight
stream on the sync ring starts immediately.
"""

import numpy as np

FRAME = 1024
ENC = 256
HID = 128
EPS = 1e-7
NCORES = 8
P = 128

# matmul dtype: "bf16" (fast, abs err ~3e-3 of scale) or "f32" (exact)
WDT = "bf16"

# gate order i, f, o, g (PyTorch layout is i, f, g, o)
_PERM = np.concatenate([np.arange(0, 128), np.arange(128, 256),
                        np.arange(384, 512), np.arange(256, 384)])

NV = 16   # f32 small-vector blob columns
NX = 10   # bf16 activation blob columns (y1 k-tiles, h1_in, h2_in)

_CACHE: dict = {}


def _build_bass():
    import concourse.bacc as bacc
    import concourse.mybir as mybir
    import concourse.tile as tile

    f32 = mybir.dt.float32
    wdt = mybir.dt.bfloat16 if WDT == "bf16" else f32
    AFT = mybir.ActivationFunctionType
    ALU = mybir.AluOpType

    nc = bacc.Bacc()

    d_vecs = nc.dram_tensor("vecs", [16, P + 16], wdt, kind="ExternalInput")
    d_xq = nc.dram_tensor("xq", [16, P + 16], wdt, kind="ExternalInput")
    d_we0 = nc.dram_tensor("we0", [P, 1024], wdt, kind="ExternalInput")
    d_we1 = nc.dram_tensor("we1", [P, 1024], wdt, kind="ExternalInput")
    d_wz1 = nc.dram_tensor("wz1", [P, 1536], wdt, kind="ExternalInput")
    d_wz2 = nc.dram_tensor("wz2", [P, 1024], wdt, kind="ExternalInput")
    d_wtl = nc.dram_tensor("wtl", [P, 512 + P], wdt, kind="ExternalInput")
    d_hc = nc.dram_tensor("hc", [4, P], f32, kind="ExternalOutput")
    d_dec = nc.dram_tensor("dec", [1, P], f32, kind="ExternalOutput")

    with tile.TileContext(nc) as tc:
        with (
            tc.tile_pool(name="w", bufs=1) as wp,
            tc.tile_pool(name="s", bufs=1) as sp,
            tc.tile_pool(name="ps", bufs=1, space="PSUM") as pp,
        ):
            vecs_r = wp.tile([16, P + 16], wdt, tag="vecs_r")
            xq_r = wp.tile([16, P + 16], wdt, tag="xq_r")
            we0 = wp.tile([P, 1024], wdt, tag="we0")
            we1 = wp.tile([P, 1024], wdt, tag="we1")
            wz1 = wp.tile([P, 1536], wdt, tag="wz1")
            wz2 = wp.tile([P, 1024], wdt, tag="wz2")
            wtl = wp.tile([P, 512 + P], wdt, tag="wtl")

            # Small inputs land ROW-major on the ACT HWDGE ring (few fat
            # descriptors instead of 128 tiny ones, which would both
            # trickle completions and steal SDMA attention from the
            # weight stream), then are transposed on-chip by the PE using
            # a 16x16 identity embedded in rows 16:32 of each blob.
            nc.sync.dma_start(xq_r[:], d_xq[:])
            nc.sync.dma_start(vecs_r[:], d_vecs[:])
            nc.sync.dma_start(we0[:], d_we0[:])
            nc.sync.dma_start(we1[:], d_we1[:])
            nc.sync.dma_start(wz1[:], d_wz1[:])
            nc.sync.dma_start(wz2[:], d_wz2[:])
            nc.sync.dma_start(wtl[:], d_wtl[:])

            ones = sp.tile([P, P], wdt, tag="ones")
            nc.gpsimd.memset(ones[:], 1.0)
            konst = sp.tile([1, 1], f32, tag="konst")
            nc.gpsimd.memset(konst[:], 1.0)
            junk1 = sp.tile([1, 1], f32, tag="junk1")
            # Preload the Sqrt table set at t=0 (hidden under DMA); only
            # one ACT table set is resident at a time, so the Sigmoid set
            # is loaded right after the real LN sqrt (under the LSTM1 MMs).
            nc.scalar.activation(junk1[:], konst[:], AFT.Sqrt)

            # on-chip transpose of the small input blobs (PE idle here)
            xqT_ps = pp.tile([P, 16], f32, tag="v2_ps")
            nc.tensor.matmul(xqT_ps[:], xq_r[0:16, 0:P], xq_r[0:16, P:P + 16],
                             start=True, stop=True)
            xq = sp.tile([P, 16], wdt, tag="xq")
            with nc.allow_low_precision("bf16 xq transpose"):
                nc.vector.tensor_copy(xq[:], xqT_ps[:])
            vecsT_ps = pp.tile([P, 16], f32, tag="u2_ps")
            nc.tensor.matmul(vecsT_ps[:], vecs_r[0:16, 0:P],
                             vecs_r[0:16, P:P + 16], start=True, stop=True)
            vecs = sp.tile([P, 16], f32, tag="vecs")
            nc.vector.tensor_copy(vecs[:], vecsT_ps[:])

            # ---- encoder: enc[256] = enc_W @ y1, columns of enc_ps ----
            enc_ps = pp.tile([P, 2], f32, tag="enc_ps")
            for m in range(2):
                for kk in range(8):
                    w = we0 if kk < 4 else we1
                    c = (kk % 4) * 256 + 128 * m
                    nc.tensor.matmul(enc_ps[:, m:m + 1], w[:, c:c + 128],
                                     xq[:, kk:kk + 1],
                                     start=(kk == 0), stop=(kk == 7))
            enc_sb = sp.tile([P, 2], f32, tag="enc_sb")
            if WDT == "bf16":
                enc_bf = sp.tile([P, 2], wdt, tag="enc_bf")
                nc.vector.tensor_copy(enc_bf[:], enc_ps[:])
            nc.vector.tensor_copy(enc_sb[:], enc_ps[:])
            if WDT != "bf16":
                enc_bf = enc_sb

            # ---- LN statistics: sum and sum-of-squares via ones-matmul ----
            sq = sp.tile([P, 2], wdt, tag="sq")
            with nc.allow_low_precision("bf16 LN stats"):
                nc.vector.tensor_mul(sq[:], enc_bf[:], enc_bf[:])
            st_ps = pp.tile([P, 2], f32, tag="st_ps")
            for kk in range(2):
                nc.tensor.matmul(st_ps[:, 0:1], ones[:], enc_bf[:, kk:kk + 1],
                                 start=(kk == 0), stop=(kk == 1))
            for kk in range(2):
                nc.tensor.matmul(st_ps[:, 1:2], ones[:], sq[:, kk:kk + 1],
                                 start=(kk == 0), stop=(kk == 1))
            ms = sp.tile([P, 2], f32, tag="ms")  # [mean, E[x^2]]
            nc.scalar.activation(ms[:], st_ps[:], AFT.Identity,
                                 scale=1.0 / ENC)
            mean = ms[:, 0:1]
            # m2e = mean^2 - EPS, so ve = E[x^2] - mean^2 + EPS
            m2 = sp.tile([P, 1], f32, tag="m2")
            nc.vector.tensor_scalar(m2[:], ms[:, 0:1], ms[:, 0:1], -EPS,
                                    ALU.mult, ALU.add)
            ve = sp.tile([P, 1], f32, tag="ve")
            nc.vector.tensor_sub(ve[:], ms[:, 1:2], m2[:])
            std = sp.tile([P, 1], f32, tag="std")
            nc.scalar.activation(std[:], ve[:], AFT.Sqrt)
            # kick the Sigmoid/Tanh table load now, under the LSTM1 MMs
            junk2 = sp.tile([1, 1], f32, tag="junk2")
            nc.scalar.activation(junk2[:], std[0:1, 0:1], AFT.Sigmoid)
            rstd = sp.tile([P, 1], f32, tag="rstd")
            nc.vector.reciprocal(rstd[:], std[:])
            ns = sp.tile([P, 1], f32, tag="ns")
            nc.vector.tensor_scalar(ns[:], ms[:, 0:1], rstd[:], -1.0,
                                    ALU.mult, ALU.mult)
            bias1 = sp.tile([P, 4], f32, tag="bias1")
            nc.vector.tensor_scalar(bias1[:], vecs[:, 8:12], ns[:], None,
                                    ALU.mult)
            bias1b = sp.tile([P, 4], f32, tag="bias1b")
            nc.vector.tensor_tensor(bias1b[:], bias1[:], vecs[:, 0:4],
                                    ALU.add)

            # ---- LSTM1: u1 = Wih1g@enc (raw enc!), v1 = Whh1@h1_in ----
            u1_ps = pp.tile([P, 4], f32, tag="u1_ps")
            for m in range(4):
                for kk in range(2):
                    c = 512 * kk + 128 * m
                    nc.tensor.matmul(u1_ps[:, m:m + 1], wz1[:, c:c + 128],
                                     enc_bf[:, kk:kk + 1],
                                     start=(kk == 0), stop=(kk == 1))
            v1_ps = pp.tile([P, 4], f32, tag="v1_ps")
            for m in range(4):
                c = 1024 + 128 * m
                nc.tensor.matmul(v1_ps[:, m:m + 1], wz1[:, c:c + 128],
                                 xq[:, 8:9], start=True, stop=True)
            v1b = sp.tile([P, 4], f32, tag="v1b")
            nc.vector.tensor_tensor(v1b[:], v1_ps[:], bias1b[:], ALU.add)
            t0 = sp.tile([P, 4], f32, tag="t0")
            nc.vector.tensor_scalar(t0[:], u1_ps[:], rstd[:], None, ALU.mult)
            zin1 = sp.tile([P, 4], f32, tag="zin1")
            nc.vector.tensor_tensor(zin1[:], t0[:], v1b[:], ALU.add)
            g1 = sp.tile([P, 4], f32, tag="g1")
            nc.scalar.activation(g1[:, 0:3], zin1[:, 0:3], AFT.Sigmoid)
            nc.scalar.activation(g1[:, 3:4], zin1[:, 3:4], AFT.Tanh)

            hc = sp.tile([P, 4], f32, tag="hc")  # h1, c1, h2, c2 columns
            p1 = sp.tile([P, 1], f32, tag="p1")
            nc.vector.tensor_mul(p1[:], g1[:, 0:1], g1[:, 3:4])
            # c1_new = c1_in * f + i*g
            nc.vector.tensor_scalar(hc[:, 1:2], vecs[:, 4:5], g1[:, 1:2],
                                    p1[:], ALU.mult, ALU.add)
            tc1 = sp.tile([P, 1], f32, tag="tc1")
            nc.scalar.activation(tc1[:], hc[:, 1:2], AFT.Tanh)
            if WDT == "bf16":
                h1_bf = sp.tile([P, 1], wdt, tag="h1_bf")
                with nc.allow_low_precision("bf16 h1 for matmul rhs"):
                    nc.vector.tensor_mul(h1_bf[:], g1[:, 2:3], tc1[:])
                h1_x = h1_bf[:]
            nc.vector.tensor_mul(hc[:, 0:1], g1[:, 2:3], tc1[:])
            if WDT != "bf16":
                h1_x = hc[:, 0:1]

            # ---- LSTM2: v2 = Whh2@h2_in runs early; u2 = Wih2@h1 ----
            v2_ps = pp.tile([P, 4], f32, tag="v2_ps")
            for m in range(4):
                c = 512 + 128 * m
                nc.tensor.matmul(v2_ps[:, m:m + 1], wz2[:, c:c + 128],
                                 xq[:, 9:10], start=True, stop=True)
            e2 = sp.tile([P, 4], f32, tag="e2")
            nc.vector.tensor_tensor(e2[:], v2_ps[:], vecs[:, 12:16], ALU.add)
            u2_ps = pp.tile([P, 4], f32, tag="u2_ps")
            for m in range(4):
                nc.tensor.matmul(u2_ps[:, m:m + 1],
                                 wz2[:, 128 * m:128 * m + 128],
                                 h1_x, start=True, stop=True)
            zin2 = sp.tile([P, 4], f32, tag="zin2")
            nc.vector.tensor_tensor(zin2[:], u2_ps[:], e2[:], ALU.add)
            g2 = sp.tile([P, 4], f32, tag="g2")
            nc.scalar.activation(g2[:, 0:3], zin2[:, 0:3], AFT.Sigmoid)
            nc.scalar.activation(g2[:, 3:4], zin2[:, 3:4], AFT.Tanh)
            p2 = sp.tile([P, 1], f32, tag="p2")
            nc.vector.tensor_mul(p2[:], g2[:, 0:1], g2[:, 3:4])
            nc.vector.tensor_scalar(hc[:, 3:4], vecs[:, 5:6], g2[:, 1:2],
                                    p2[:], ALU.mult, ALU.add)
            tc2 = sp.tile([P, 1], f32, tag="tc2")
            nc.scalar.activation(tc2[:], hc[:, 3:4], AFT.Tanh)
            if WDT == "bf16":
                h2_bf = sp.tile([P, 1], wdt, tag="h2_bf")
                with nc.allow_low_precision("bf16 h2 for matmul rhs"):
                    nc.vector.tensor_mul(h2_bf[:], g2[:, 2:3], tc2[:])
                h2_x = h2_bf[:]
            nc.vector.tensor_mul(hc[:, 2:3], g2[:, 2:3], tc2[:])
            if WDT != "bf16":
                h2_x = hc[:, 2:3]

            # ---- dense mask + decoder shard ----
            d_ps = pp.tile([P, 2], f32, tag="enc_ps")
            for m in range(2):
                nc.tensor.matmul(d_ps[:, m:m + 1],
                                 wtl[:, 128 * m:128 * m + 128],
                                 h2_x, start=True, stop=True)
            msk = sp.tile([P, 2], f32, tag="msk")
            for m in range(2):
                nc.scalar.activation(msk[:, m:m + 1], d_ps[:, m:m + 1],
                                     AFT.Sigmoid, bias=vecs[:, 6 + m:7 + m])
            if WDT == "bf16":
                est_bf = sp.tile([P, 2], wdt, tag="est_bf")
                with nc.allow_low_precision("bf16 est for matmul rhs"):
                    nc.vector.tensor_mul(est_bf[:], msk[:], enc_sb[:])
            else:
                est_bf = sp.tile([P, 2], f32, tag="est_bf")
                nc.vector.tensor_mul(est_bf[:], msk[:], enc_sb[:])
            # operand-swapped decoder matvec: out is a row [1, 128]
            o_ps = pp.tile([1, P], f32, tag="u1_ps")
            for kk in range(2):
                c = 256 + 128 * kk
                nc.tensor.matmul(o_ps[:], est_bf[:, kk:kk + 1],
                                 wtl[:, c:c + 128],
                                 start=(kk == 0), stop=(kk == 1))
            dec_sb = sp.tile([1, P], f32, tag="dec_sb")
            nc.vector.tensor_copy(dec_sb[:], o_ps[:])
            nc.scalar.dma_start(d_dec[:], dec_sb[:])

            # transpose h/c to rows via identity matmul: [4,128] out
            # (emitted AFTER the dec chain so it doesn't delay the
            # critical dense->mask->dec path on the PE)
            if WDT == "bf16":
                hc_w = sp.tile([P, 4], wdt, tag="hc_w")
                with nc.allow_low_precision("bf16 hc transpose"):
                    nc.vector.tensor_copy(hc_w[:], hc[:])
            else:
                hc_w = hc
            hcT_ps = pp.tile([4, P], f32, tag="st_ps")
            nc.tensor.matmul(hcT_ps[:], hc_w[:, 0:4], wtl[:, 512:512 + P],
                             start=True, stop=True)
            hcT = sp.tile([4, P], f32, tag="hcT")
            nc.vector.tensor_copy(hcT[:], hcT_ps[:])
            nc.sync.dma_start(d_hc[:], hcT[:])

    nc.compile()
    return nc


def _np_wdt():
    if WDT == "bf16":
        import ml_dtypes
        return ml_dtypes.bfloat16
    return np.float32


def _pack_inputs(inputs):
    """Host-side packing: transpose/permute weights into lhsT tile blobs."""
    f = lambda x: np.ascontiguousarray(np.asarray(x, dtype=np.float32))
    y1 = f(inputs["y1"])
    h1_in, c1_in = f(inputs["h1_in"]), f(inputs["c1_in"])
    h2_in, c2_in = f(inputs["h2_in"]), f(inputs["c2_in"])
    enc_W = f(inputs["enc_W"])
    gamma, beta = f(inputs["gamma"]), f(inputs["beta"])
    Wih1, Whh1 = f(inputs["Wih1"]), f(inputs["Whh1"])
    bih1, bhh1 = f(inputs["bih1"]), f(inputs["bhh1"])
    Wih2, Whh2 = f(inputs["Wih2"]), f(inputs["Whh2"])
    bih2, bhh2 = f(inputs["bih2"]), f(inputs["bhh2"])
    dense_W, dense_b = f(inputs["dense_W"]), f(inputs["dense_b"])
    dec_W = f(inputs["dec_W"])
    q = _np_wdt()

    G1 = Wih1 * gamma[None, :]
    Pg1 = G1[_PERM]                       # [512, 256] gate-permuted
    Ph1 = Whh1[_PERM]
    Pi2 = Wih2[_PERM]
    Ph2 = Whh2[_PERM]
    c1b = (Wih1 @ beta + bih1 + bhh1)[_PERM]
    c2b = (bih2 + bhh2)[_PERM]
    w1v = Pg1.sum(axis=1)                 # Wih1g @ ones

    vecs = np.zeros((16, P + 16), np.float32)
    vecs[0:4, 0:P] = c1b.reshape(4, P)
    vecs[4, 0:P] = c1_in
    vecs[5, 0:P] = c2_in
    vecs[6:8, 0:P] = dense_b.reshape(2, P)
    vecs[8:12, 0:P] = w1v.reshape(4, P)
    vecs[12:16, 0:P] = c2b.reshape(4, P)
    vecs[0:16, P:P + 16] = np.eye(16, dtype=np.float32)

    xq = np.zeros((16, P + 16), np.float32)
    xq[0:8, 0:P] = y1.reshape(8, P)
    xq[8, 0:P] = h1_in
    xq[9, 0:P] = h2_in
    xq[0:16, P:P + 16] = np.eye(16, dtype=np.float32)

    eT = np.ascontiguousarray(enc_W.T).reshape(8, P, ENC)  # k-tiles
    we0 = np.concatenate([eT[i] for i in range(4)], axis=1)
    we1 = np.concatenate([eT[i] for i in range(4, 8)], axis=1)

    g1T = np.ascontiguousarray(Pg1.T).reshape(2, P, 512)
    wz1 = np.concatenate([g1T[0], g1T[1], Ph1.T], axis=1)  # [128, 1536]
    wz2 = np.concatenate([Pi2.T, Ph2.T], axis=1)           # [128, 1024]

    in_maps = []
    for k in range(NCORES):
        Dk = dec_W[P * k:P * (k + 1), :]                   # [128, 256]
        dT = np.ascontiguousarray(Dk.T).reshape(2, P, P)
        wtl = np.concatenate([dense_W.T, dT[0], dT[1],
                              np.eye(P, dtype=np.float32)], axis=1)
        in_maps.append({
            "vecs": np.ascontiguousarray(vecs.astype(q)),
            "xq": np.ascontiguousarray(xq.astype(q)),
            "we0": np.ascontiguousarray(we0.astype(q)),
            "we1": np.ascontiguousarray(we1.astype(q)),
            "wz1": np.ascontiguousarray(wz1.astype(q)),
            "wz2": np.ascontiguousarray(wz2.astype(q)),
            "wtl": np.ascontiguousarray(wtl.astype(q)),
        })
    return in_maps


def _get_nc():
    if "nc" not in _CACHE:
        _CACHE["nc"] = _build_bass()
    return _CACHE["nc"]


def kernel(**inputs):
    from concourse.bass_utils import run_bass_kernel_spmd

    nc = _get_nc()
    in_maps = _pack_inputs(inputs)
    res = run_bass_kernel_spmd(nc, in_maps, list(range(NCORES))).results

    decoded = np.concatenate([res[k]["dec"][0, :] for k in range(NCORES)])
    hc = res[0]["hc"]
    return (
        decoded.reshape(1, FRAME, 1).astype(np.float32),
        hc[0].reshape(1, 1, HID).astype(np.float32),
        hc[1].reshape(1, 1, HID).astype(np.float32),
        hc[2].reshape(1, 1, HID).astype(np.float32),
        hc[3].reshape(1, 1, HID).astype(np.float32),
    )
